# revision 60
# baseline (speedup 1.0000x reference)
"""DeepseekV2 decoder layer on 8 trn2 NeuronCores.

Sharding: core c -> batch b=c//4, seq-shard j=c%4 (strided 128-token chunks
{j, 4+j, 8+j, 12+j} of the 2048-token sequence).  Each core computes the
full layer for its 512 query tokens; the compressed-kv stream (kv_a, kv_b)
is computed for all 2048 tokens on every core (replicated), so no
cross-core communication is needed.  Host code shards inputs / gathers
outputs and folds all layernorm weights + rope deinterleave permutations
into the weight matrices.

Attention runs in transposed-softmax layout: scores are computed as
S^T [keys, queries], exp'd without max subtraction (scores are bounded),
the denominator comes from a ones-matmul partition reduce, and AV
consumes the probabilities directly - no probability transposes.
Most operands are bf16 (fp32 PSUM accumulation everywhere).
"""

import os
import sys
import functools
import numpy as np
import ml_dtypes

for _p in ("/opt/trn_rl_repo", "/root/.axon_site/_ro/trn_rl_repo"):
    if os.path.isdir(_p) and _p not in sys.path:
        sys.path.insert(0, _p)
os.environ.setdefault("MYCRO_LOCAL_CACHE", "1")

B, S, H = 2, 2048, 2048
NH = 16
QLR, KVLR = 1536, 512
ROPE, NOPE, VD = 64, 128, 128
QHD = NOPE + ROPE  # 192
IM = 10944
IMP = 11008  # padded to 86*128
EPS = 1e-6
P = 128
TQ = 512  # query tokens per core
SCALE = float(QHD) ** -0.5
NEG = -1.0e9

# span start (in q chunks of 128) for each key chunk kc; kc 12-15 padded
# from 3 to 2 so every score matmul has free size >= 256
FQ = [0, 0, 0, 0, 1, 1, 1, 1, 2, 2, 2, 2, 2, 2, 2, 2]
# number of 128-q-chunks of mask to add for each kc (starting at FQ[kc])
MW = [1] * 12 + [2] * 4


# ---------------------------------------------------------------------------
# module builder
# ---------------------------------------------------------------------------

@functools.lru_cache(maxsize=1)
def _build():
    from contextlib import ExitStack

    import concourse.bass as bass  # noqa: F401
    from concourse import bacc, mybir, tile
    from concourse.masks import make_identity

    f32 = mybir.dt.float32
    bf16 = mybir.dt.bfloat16
    fr = mybir.dt.float32r
    AF = mybir.ActivationFunctionType
    AX = mybir.AxisListType
    OP = mybir.AluOpType

    nc = bacc.Bacc(None, target_bir_lowering=False, debug=False)

    def di(name, shape, dt=f32):
        return nc.dram_tensor(name, list(shape), dt, kind="ExternalInput").ap()

    hidb = di("hidb", (S, H), bf16)
    xqb = di("xqb", (TQ, H), bf16)
    xq = di("xq", (TQ, H))
    wqa = di("wqa", (H, QLR), bf16)
    wqb = di("wqb", (QLR, NH * QHD), bf16)     # nope h-major | pe deint
    wkva = di("wkva", (H, KVLR + 2 * ROPE), bf16)  # rope cols deint + dup
    wk = di("wk", (KVLR, NH * NOPE), bf16)
    wv = di("wv", (KVLR, NH * VD), bf16)
    wo = di("wo", (NH * VD, H), bf16)
    wg = di("wg", (H, IMP), bf16)
    wu = di("wu", (H, IMP), bf16)
    wd = di("wd", (IMP, H), bf16)
    cosq = di("cosq", (P, TQ), bf16)
    sinq = di("sinq", (P, TQ), bf16)
    cosk = di("cosk", (P, S), bf16)
    sink = di("sink", (P, S), bf16)
    maskt = di("maskt", (P, 16 * 256))
    rmat = di("rmat", (P, P))
    rmatb = di("rmatb", (P, P), bf16)
    out = nc.dram_tensor("out", [TQ, H], f32, kind="ExternalOutput").ap()

    with tile.TileContext(nc) as tc, ExitStack() as ctx:
        def dmaf(o, i):
            nc.sync.dma_start(o.bitcast(fr), i.bitcast(fr))

        # ---------------- global pools ------------------------------
        const = ctx.enter_context(tc.tile_pool(name="const", bufs=1))
        ident = const.tile([P, P], f32, name="ident")
        make_identity(nc, ident)
        identb = const.tile([P, P], bf16, name="identb")
        nc.any.tensor_copy(identb[:], ident[:])
        onest = const.tile([P, 1], f32, name="onest")
        nc.vector.memset(onest[:], 1.0)
        ones = const.tile([P, 1], f32, name="ones")
        nc.any.tensor_copy(ones[:].bitcast(fr), onest[:])
        onesb = const.tile([P, 1], bf16, name="onesb")
        nc.any.tensor_copy(onesb[:], onest[:])
        epst = const.tile([P, 1], f32, name="epst")
        nc.vector.memset(epst[:], EPS)
        rmat_s = const.tile([P, P], f32, name="rmat_s")
        dmaf(rmat_s[:], rmat)
        rmatb_s = const.tile([P, P], bf16, name="rmatb_s")
        nc.sync.dma_start(rmatb_s[:], rmatb)

        ppA = ctx.enter_context(tc.tile_pool(name="ppA", bufs=4, space="PSUM"))
        ppB = ctx.enter_context(tc.tile_pool(name="ppB", bufs=2, space="PSUM"))
        ppT = ctx.enter_context(tc.tile_pool(name="ppT", bufs=2, space="PSUM"))

        def pa(n=1, w=512):
            ts = [ppA.tile([P, w], f32, tag="a", name="pa") for _ in range(n)]
            return ts if n > 1 else ts[0]

        def pb(n=1, w=512):
            ts = [ppB.tile([P, w], f32, tag="b", name="pb") for _ in range(n)]
            return ts if n > 1 else ts[0]

        def pt_(w=512, dt=f32):
            return ppT.tile([P, w], dt, tag="t", name="pt")

        def mm(out_, lhsT, rhs, start, stop):
            nc.tensor.matmul(out_, lhsT.bitcast(fr), rhs.bitcast(fr),
                             start=start, stop=stop)

        def mmb(out_, lhsT, rhs, start, stop, skip=False):
            nc.tensor.matmul(out_, lhsT, rhs, start=start, stop=stop,
                             skip_group_check=skip)

        def rms_scale(pool, dst_scale, src, n, tagp):
            """dst_scale[P,1] = 1/sqrt(mean(src^2, free) + eps) (per row).
            Squares run in chunks of <=512 to bound scratch size."""
            rows, free = src.shape
            nch = (free + 511) // 512
            m1 = pool.tile([rows, 1], f32, tag=tagp + "m", name=tagp + "m")
            for c in range(nch):
                w = min(512, free - c * 512)
                sq = pool.tile([P, 512], f32, tag=tagp + "sq",
                               name=tagp + "sq", bufs=2)
                mp = pool.tile([rows, 1], f32, tag=tagp + "mp",
                               name=tagp + "mp", bufs=2)
                nc.scalar.activation(sq[:rows, :w],
                                     src[:, c * 512:c * 512 + w],
                                     AF.Square, accum_out=mp[:])
                if c == 0:
                    nc.vector.tensor_copy(m1[:], mp[:])
                else:
                    nc.vector.tensor_add(m1[:], m1[:], mp[:])
            srt = pool.tile([rows, 1], f32, tag=tagp + "r", name=tagp + "r")
            nc.scalar.activation(srt[:], m1[:], AF.Sqrt, scale=1.0 / n,
                                 bias=epst[:rows, :])
            nc.vector.reciprocal(dst_scale, srt[:])

        def tr4b(dst, srcs):
            """Transpose up to 4 [128, w<=128] bf16 tiles into one psum
            bank, evict with a single copy."""
            ps = pt_(512, bf16)
            npart = srcs[0].shape[-1]
            for k, s in enumerate(srcs):
                nc.tensor.matmul(ps[:npart, k * P:(k + 1) * P],
                                 s, identb[:], is_transpose=True,
                                 skip_group_check=True)
            nc.any.tensor_copy(dst, ps[:npart, :len(srcs) * P])

        def tr4(dst, srcs, rnd=True):
            ps = pt_()
            npart = srcs[0].shape[-1]
            for k, s in enumerate(srcs):
                nc.tensor.matmul(ps[:npart, k * P:(k + 1) * P],
                                 s, ident[:], is_transpose=True,
                                 skip_group_check=True)
            if rnd:
                dst = dst.bitcast(fr)
            nc.any.tensor_copy(dst, ps[:npart, :len(srcs) * P])

        # staggered-lifetime pools
        s_kvb = ExitStack()      # ckvt bf16: lives until end of attention
        s_qtp = ExitStack()      # qT / QTpe2: until last head's scores
        s_att = ExitStack()      # attnT: until o-proj
        s_c = ExitStack()        # acc: o-proj..end
        p_kvb = s_kvb.enter_context(tc.tile_pool(name="p_kvb", bufs=1))
        ckvt = p_kvb.tile([P, 5, S], bf16, name="ckvt")
        sbcl = p_kvb.tile([P, S], bf16, name="sbcl")
        sclT = p_kvb.tile([P, 16], f32, name="sclT")
        p_qtp = s_qtp.enter_context(
            tc.tile_pool(name="p_qtp", bufs=1, side="right"))
        qT = p_qtp.tile([P, NH, TQ], bf16, name="qT")
        QTpe2 = p_qtp.tile([P, 8, TQ], bf16, name="QTpe2")

        if True:
            # ========================================================
            # A1: xq -> xqt (bf16) + q-token rms scales
            # ========================================================
            s_xq = ExitStack()
            p_xq = s_xq.enter_context(tc.tile_pool(name="p_xq", bufs=1))
            xqt = p_xq.tile([P, 16, TQ], bf16, name="xqt")
            sqq = [p_xq.tile([P, 1], f32, tag="sqq%d" % t,
                             name="sqq%d" % t) for t in range(4)]
            s_sr = ExitStack()
            p_sr = s_sr.enter_context(tc.tile_pool(name="p_sr", bufs=1))
            sr = p_sr.tile([1, S], f32, name="sr")
            sr2 = p_sr.tile([1, S], f32, name="sr2")
            msum = p_sr.tile([P, 16], f32, name="msum")
            s_ckacc = ExitStack()
            p_cka = s_ckacc.enter_context(
                tc.tile_pool(name="p_cka", bufs=1))
            ckpe = p_cka.tile([P, S], f32, name="ckpe")
            s_kvt = ExitStack()
            p_kvt = s_kvt.enter_context(tc.tile_pool(name="p_kvt", bufs=2))
            ckb = p_kvt.tile([P, S], bf16, name="ckb", bufs=1)
            skb = p_kvt.tile([P, S], bf16, name="skb", bufs=1)
            nc.sync.dma_start(ckb[:], cosk)
            nc.sync.dma_start(skb[:], sink)
            s_cka2 = ExitStack()
            p_cka2 = s_cka2.enter_context(
                tc.tile_pool(name="p_cka2", bufs=1))
            ckacc = p_cka2.tile([P, 4, S], f32, name="ckacc")
            with tc.tile_pool(name="p_xs", bufs=2) as p_xs:
                for t in range(4):
                    nat = p_xs.tile([P, 4, 512], bf16, tag="natq",
                                    name="natq")
                    nc.sync.dma_start(
                        nat[:], xqb[t * P:(t + 1) * P, :].rearrange(
                            "p (c f) -> p c f", f=512))
                    m1 = p_xs.tile([P, 1], f32, tag="m1q", name="m1q")
                    for hf in range(4):
                        sq = p_xs.tile([P, 512], f32, tag="sqxq",
                                       name="sqxq")
                        mp = p_xs.tile([P, 1], f32, tag="mpq", name="mpq")
                        nc.scalar.activation(sq[:], nat[:, hf, :], AF.Square,
                                             accum_out=mp[:])
                        if hf == 0:
                            nc.vector.tensor_copy(m1[:], mp[:])
                        else:
                            nc.vector.tensor_add(m1[:], m1[:], mp[:])
                        tr4b(xqt[:, hf * 4:(hf + 1) * 4, t * P:(t + 1) * P],
                             [nat[:, hf, k * P:(k + 1) * P]
                              for k in range(4)])
                    srt = p_xs.tile([P, 1], f32, tag="srtq", name="srtq")
                    nc.scalar.activation(srt[:], m1[:], AF.Sqrt,
                                         scale=1.0 / H, bias=epst[:])
                    nc.vector.reciprocal(sqq[t][:], srt[:])

            # ========================================================
            # KV: hid -> X^T (bf16) -> ckv^T(f32); token-rms; kvlr-rms;
            #     rope k_pe; cast ckvt -> bf16
            # ========================================================
            with tc.tile_pool(name="p_kv1", bufs=1) as p_kv1, \
                 tc.tile_pool(name="p_kvs", bufs=2) as p_kvs:
                hidr = hidb.rearrange("(c p) f -> p c f", p=P)
                wkvr = wkva.rearrange("(c p) f -> p c f", p=P)
                psd = None
                for hf in range(4):
                    xt = p_kv1.tile([P, 4, S], bf16, tag="xt", name="xt")
                    wkv = p_kv1.tile([P, 4, KVLR + 2 * ROPE], bf16,
                                     tag="wkv", name="wkv")
                    nc.sync.dma_start(wkv[:], wkvr[:, 4 * hf:4 * hf + 4, :])
                    xh = p_kv1.tile([P, 16, 512], bf16, tag="xh", name="xh")
                    for tq in range(4):  # quartered so t-loop starts early
                        nc.sync.dma_start(
                            xh[:, 4 * tq:4 * tq + 4, :],
                            hidr[:, 4 * tq:4 * tq + 4,
                                 hf * 512:(hf + 1) * 512])
                    for t in range(16):
                        sq = p_kvs.tile([P, 512], f32, tag="sqh", name="sqh")
                        m1 = p_kvs.tile([P, 1], f32, tag="m1h", name="m1h")
                        nc.scalar.activation(sq[:], xh[:, t, :], AF.Square,
                                             accum_out=m1[:])
                        if hf == 0:
                            nc.vector.tensor_copy(msum[:, t:t + 1], m1[:])
                        else:
                            nc.vector.tensor_add(msum[:, t:t + 1],
                                                 msum[:, t:t + 1], m1[:])
                        tr4b(xt[:, :, t * P:(t + 1) * P],
                             [xh[:, t, k * P:(k + 1) * P] for k in range(4)])
                    if hf == 3:
                        # token sumsq row (overlaps the final cc matmuls):
                        # msum [128,16] -> sr [1, 2048]
                        pst = pt_()
                        nc.tensor.transpose(pst[0:16, :128], msum[:],
                                            ident[:])
                        t16 = p_kvs.tile([16, P], f32, tag="t16",
                                         name="t16")
                        nc.scalar.copy(t16[:], pst[0:16, :128])
                        nc.sync.dma_start(sr[0:1, :], t16[:])
                    for cc in range(5):
                        pk = pa(4)
                        for hcl in range(4):
                            for g in range(4):
                                mmb(pk[g], wkv[:, hcl, cc * P:(cc + 1) * P],
                                    xt[:, hcl, g * 512:(g + 1) * 512],
                                    hcl == 0, hcl == 3)
                        for g in range(4):
                            gsl = slice(g * 512, (g + 1) * 512)
                            dst = (ckpe[:, gsl] if cc == 4
                                   else ckacc[:, cc, gsl])
                            if hf == 0:
                                nc.scalar.copy(dst.bitcast(fr), pk[g])
                            elif hf < 3 or cc == 4:
                                nc.any.tensor_add(dst.bitcast(fr), dst,
                                                  pk[g])
                            else:
                                # final accumulation writes RAW bf16 ckv
                                # directly (scales are folded into the
                                # K/V builds in the attention phase)
                                nc.any.tensor_add(ckvt[:, cc, gsl], dst,
                                                  pk[g])
                # kvlr sumsq of the raw ckv chunks (partition reduce)
                psd = [ppB.tile([P, 512], f32, tag="b", name="psd"),
                       ppB.tile([P, 512], f32, tag="b", name="psd"),
                       ppT.tile([P, 512], f32, tag="t", name="psd"),
                       ppT.tile([P, 512], f32, tag="t", name="psd")]
                for cc in range(4):
                    for g in range(4):
                        sq = p_kvs.tile([P, 512], f32, tag="sqckv",
                                        name="sqckv")
                        nc.scalar.activation(
                            sq[:].bitcast(fr),
                            ckvt[:, cc, g * 512:(g + 1) * 512], AF.Square)
                        mm(psd[g][0:1, :], ones[:], sq[:],
                           cc == 0, cc == 3)
            s_cka2.close()

            # ========================================================
            # KV tail: per-token scale rows (folded into consumers),
            # rope k_pe.  Short serial chain; Q path overlaps on PE.
            # ========================================================
            if True:
                sc2 = p_kvt.tile([1, S], f32, name="sc2", bufs=1)
                # s_tok = rsqrt(msum/H + eps)
                nc.scalar.activation(sr2[:], sr[:], AF.Sqrt, scale=1.0 / H,
                                     bias=epst[0:1, :])
                nc.vector.reciprocal(sr[:], sr2[:])
                # kvlr sumsq row (raw) -> sc2
                for g in range(4):
                    nc.scalar.copy(sc2[0:1, g * 512:(g + 1) * 512],
                                   psd[g][0:1, :])
                # s_kv = rsqrt(sumsq_raw*s_tok^2/KVLR + eps);
                # sc_low = s_tok*s_kv  (into sc2); sc_pe = s_tok (sr)
                nc.vector.tensor_mul(sr2[:], sr[:], sr[:])
                nc.vector.tensor_mul(sc2[:], sc2[:], sr2[:])
                nc.scalar.activation(sc2[:], sc2[:], AF.Sqrt,
                                     scale=1.0 / KVLR, bias=epst[0:1, :])
                nc.vector.reciprocal(sc2[:], sc2[:])
                nc.vector.tensor_mul(sc2[:], sc2[:], sr[:])
                # broadcast sc_low along partitions (bf16, for kt evict
                # muls) and transpose to [128,16] (for v4 evict scalars)
                sc2b = p_kvt.tile([1, S], bf16, name="sc2b", bufs=1)
                nc.any.tensor_copy(sc2b[:], sc2[:])
                nc.gpsimd.partition_broadcast(sbcl[:], sc2b[0:1, :])
                t16b = p_kvt.tile([16, P], f32, name="t16b", bufs=1)
                nc.sync.dma_start(t16b[:], sc2[0:1, :])
                pst2 = pt_()
                nc.tensor.transpose(pst2[0:P, 0:16], t16b[:],
                                    ident[0:16, 0:16])
                nc.scalar.copy(sclT[:], pst2[0:P, 0:16])
                # rope k_pe (chunk 4, duplicated halves); s_tok is
                # applied to kp first (commutes with the rotation)
                sbc = p_kvt.tile([P, S], f32, name="sbc", bufs=1)
                nc.gpsimd.partition_broadcast(sbc[:], sr[0:1, :])
                nc.vector.tensor_mul(ckpe[:].bitcast(fr), ckpe[:], sbc[:])
                for g in range(4):
                    sl = slice(g * 512, (g + 1) * 512)
                    kp = ckpe[:, sl]
                    psw = pb()
                    mm(psw[:], rmat_s[:], kp, True, True)
                    t1 = p_kvt.tile([P, 512], f32, tag="krt1", name="krt1")
                    t2 = p_kvt.tile([P, 512], f32, tag="krt2", name="krt2")
                    nc.vector.tensor_mul(t1[:], kp, ckb[:, sl])
                    nc.vector.tensor_mul(t2[:], psw[:], skb[:, sl])
                    nc.vector.tensor_add(ckvt[:, 4, sl], t1[:], t2[:])
            s_kvt.close()
            s_ckacc.close()
            s_sr.close()

            # ========================================================
            # Q path: q_a (token-major, rms) -> qanT (bf16);
            # q_b^T direct -> qT (nope) + QTpe2 (rope, 2 heads/slot);
            # rope q
            # ========================================================
            with tc.tile_pool(name="p_q", bufs=1) as p_q, \
                 tc.tile_pool(name="p_qs", bufs=2) as p_qs, \
                 tc.high_priority(offset=2000):
                qanT = p_q.tile([P, 12, TQ], bf16, name="qanT")
                cq = p_q.tile([P, TQ], bf16, name="cq")
                sq_ = p_q.tile([P, TQ], bf16, name="sq_")
                nc.sync.dma_start(cq[:], cosq)
                nc.sync.dma_start(sq_[:], sinq)
                wqar = wqa.rearrange("(c p) f -> p c f", p=P)
                with tc.tile_pool(name="p_qa", bufs=2) as p_qa:
                    qa_t = [p_qa.tile([P, QLR], f32, tag="qanat%d" % t,
                                      name="qanat%d" % t, bufs=1)
                            for t in range(4)]
                    for f in range(3):
                        w = p_qa.tile([P, 16, 512], bf16, tag="wqat",
                                      name="wqat")
                        nc.sync.dma_start(
                            w[:], wqar[:, :, f * 512:(f + 1) * 512])
                        for tp in range(2):  # 2 psum banks at a time
                            psq = pa(2)
                            for hc in range(16):
                                for ti in range(2):
                                    t = 2 * tp + ti
                                    mmb(psq[ti],
                                        xqt[:, hc, t * P:(t + 1) * P],
                                        w[:, hc, :], hc == 0, hc == 15)
                            for ti in range(2):
                                t = 2 * tp + ti
                                nc.vector.tensor_scalar_mul(
                                    qa_t[t][:, f * 512:(f + 1) * 512],
                                    psq[ti], sqq[t][:])
                    for t in range(4):
                        qa = qa_t[t]
                        s2 = p_qa.tile([P, 1], f32, tag="s2", name="s2")
                        rms_scale(p_qa, s2[:], qa[:], QLR, "qa")
                        qab = p_qa.tile([P, QLR], bf16, tag="qab",
                                        name="qab")
                        nc.vector.tensor_scalar_mul(qab[:], qa[:], s2[:])
                        for g in range(3):
                            tr4b(qanT[:, 4 * g:4 * (g + 1),
                                      t * P:(t + 1) * P],
                                 [qab[:, (4 * g + k) * P:(4 * g + k + 1) * P]
                                  for k in range(4)])
                # q_b^T: out [qhd-chunk, tok]; 24 chunks (16 nope + 8 pe)
                wqbr = wqb.rearrange("(c p) f -> p c f", p=P)
                with tc.tile_pool(name="p_qb", bufs=2) as p_qb:
                    for op in range(12):  # pairs of output chunks
                        w = p_qb.tile([P, 12, 256], bf16, tag="wqbt",
                                      name="wqbt")
                        nc.sync.dma_start(
                            w[:], wqbr[:, :, op * 256:(op + 1) * 256])
                        for oh in range(2):
                            oc = 2 * op + oh
                            po = pb()
                            for lc in range(12):
                                mmb(po, w[:, lc, oh * P:(oh + 1) * P],
                                    qanT[:, lc, :], lc == 0, lc == 11)
                            if oc < 16:
                                nc.any.tensor_copy(qT[:, oc, :], po)
                            else:
                                nc.any.tensor_copy(QTpe2[:, oc - 16, :], po)
                # rope q_pe (2 heads per slot; rmat is block-diag 2x64)
                for c in range(8):
                    qd = QTpe2[:, c, :]
                    psw = pb()
                    mmb(psw, rmatb_s[:], qd, True, True)
                    t1 = p_qs.tile([P, TQ], bf16, tag="qrt1", name="qrt1")
                    t2 = p_qs.tile([P, TQ], bf16, tag="qrt2", name="qrt2")
                    nc.vector.tensor_mul(t1[:], qd, cq[:])
                    nc.vector.tensor_mul(t2[:], psw[:], sq_[:])
                    nc.vector.tensor_add(qd, t1[:], t2[:])

            s_xq.close()

            # ========================================================
            # Attention: transposed-softmax layout
            # ========================================================
            p_at = s_att.enter_context(
                tc.tile_pool(name="p_at", bufs=1, side="right"))
            attnT = p_at.tile([P, NH, TQ], bf16, name="attnT")
            with tc.tile_pool(name="p_b1", bufs=1) as p_b1, \
                 tc.tile_pool(name="p_bs", bufs=2) as p_bs:
                wkr_ = wk.rearrange("(c p) f -> p c f", p=P)
                wvr_ = wv.rearrange("(c p) f -> p c f", p=P)
                wk_s = p_b1.tile([P, 4, NH * NOPE], bf16, name="wk_s")
                wv_s = p_b1.tile([P, 4, NH * VD], bf16, name="wv_s")
                for cc in range(4):  # split so the first kt build can
                    nc.sync.dma_start(wk_s[:, cc, :], wkr_[:, cc, :])
                for cc in range(4):
                    nc.sync.dma_start(wv_s[:, cc, :], wvr_[:, cc, :])
                masks = p_b1.tile([P, 16, 256], f32, name="masks")
                nc.sync.dma_start(
                    masks[:], maskt.rearrange("p (c f) -> p c f", f=256))
                for h in range(NH):
                    g4, hh, par = h // 4, h % 4, h % 2
                    # K^T for this head: [nope, keys]
                    kt = p_b1.tile([P, S], bf16, tag="kt", name="kt",
                                   bufs=2)
                    for g in range(4):
                        pk = pb()
                        for cc in range(4):
                            mmb(pk, wk_s[:, cc, h * NOPE:(h + 1) * NOPE],
                                ckvt[:, cc, g * 512:(g + 1) * 512],
                                cc == 0, cc == 3)
                        # eviction applies the per-key-token combined
                        # token+kvlr rms scale (commutes with W_k)
                        nc.vector.tensor_mul(kt[:, g * 512:(g + 1) * 512],
                                             pk, sbcl[:, g * 512:(g + 1) * 512])
                    # V for 4-head group: [keys, vd(4 heads)]
                    if hh == 0:
                        v4 = p_b1.tile([P, 16, 512], bf16, tag="v4",
                                       name="v4", bufs=2)
                        for kc in range(16):
                            pv = pb()
                            for cc in range(4):
                                mmb(pv,
                                    ckvt[:, cc, kc * P:(kc + 1) * P],
                                    wv_s[:, cc, g4 * 512:(g4 + 1) * 512],
                                    cc == 0, cc == 3)
                            nc.vector.tensor_scalar_mul(
                                v4[:, kc, :], pv, sclT[:, kc:kc + 1])
                    # pass 1: scores^T -> mask -> exp -> probs (bf16)
                    probs = p_b1.tile([P, 16, TQ], bf16, tag="probs",
                                      name="probs", bufs=2)
                    for kc in range(16):
                        fq = FQ[kc] * P
                        s = ppA.tile([P, TQ], f32, tag="a", name="sc")
                        mmb(s[:, fq:], kt[:, kc * P:(kc + 1) * P],
                            qT[:, h, fq:], True, False)
                        nc.tensor.matmul(
                            s[:, fq:],
                            ckvt[64 * par:64 * par + 64, 4,
                                 kc * P:(kc + 1) * P],
                            QTpe2[64 * par:64 * par + 64, h // 2, fq:],
                            start=False, stop=True)
                        mo = FQ[kc] * P
                        mwd = MW[kc] * P
                        nc.vector.tensor_add(
                            s[:, mo:mo + mwd], s[:, mo:mo + mwd],
                            masks[:, kc, 0:mwd])
                        nc.scalar.activation(probs[:, kc, fq:], s[:, fq:],
                                             AF.Exp, scale=SCALE)
                    # pass 2: AV + denominator accumulate over kc
                    pav = pt_()
                    pdn = pt_()
                    for kc in range(16):
                        fq = FQ[kc] * P
                        mmb(pav[:, fq:],
                            v4[:, kc, hh * P:(hh + 1) * P],
                            probs[:, kc, fq:], kc == 0, kc == 15, skip=True)
                        mmb(pdn[0:1, fq:], onesb[:],
                            probs[:, kc, fq:], kc == 0, kc == 15, skip=True)
                    dr = p_bs.tile([1, TQ], f32, tag="dr", name="dr")
                    nc.vector.reciprocal(dr[:], pdn[0:1, :])
                    dbc = p_bs.tile([P, TQ], f32, tag="dbc", name="dbc")
                    nc.gpsimd.partition_broadcast(dbc[:], dr[0:1, :])
                    nc.vector.tensor_mul(attnT[:, h, :], pav[:], dbc[:])
            s_kvb.close()

        # ============================================================
        # C: o-proj + residual; MLP
        # ============================================================
        p_c = s_c.enter_context(tc.tile_pool(name="p_c", bufs=1))
        acc = [p_c.tile([P, H], f32, tag="acc%d" % t, name="acc%d" % t)
               for t in range(4)]
        with tc.tile_pool(name="p_cs", bufs=1) as p_cs, \
             tc.tile_pool(name="wbig", bufs=2) as wbig:
            wgr = wg.rearrange("(c p) f -> p c f", p=P)
            wur = wu.rearrange("(c p) f -> p c f", p=P)
            wdr = wd.rearrange("(c p) f -> p c f", p=P)
            # prefetch the first two MLP gate/up weight pairs so phase 1
            # starts without DMA stalls
            pre_w = []
            for ip in range(2):
                wgt = wbig.tile([P, 16, 256], bf16, tag="wg", name="wgt")
                wut = wbig.tile([P, 16, 256], bf16, tag="wu", name="wut")
                pre_w.append((wgt, wut))
            wor = wo.rearrange("(c p) f -> p c f", p=P)
            with tc.tile_pool(name="p_co", bufs=2) as p_co:
                mq4 = [p_cs.tile([P, 4], f32, tag="mq%d" % t,
                                 name="mq%d" % t) for t in range(4)]
                for f in range(4):
                    pso = pa(4)
                    w = p_co.tile([P, 16, 512], bf16, tag="wot", name="wot")
                    nc.sync.dma_start(
                        w[:], wor[:, :, f * 512:(f + 1) * 512])
                    if f < 2:  # prefetch MLP weights behind the wo loads
                        wgt, wut = pre_w[f]
                        nc.sync.dma_start(
                            wgt[:], wgr[:, :, f * 256:(f + 1) * 256])
                        nc.sync.dma_start(
                            wut[:], wur[:, :, f * 256:(f + 1) * 256])
                    for hc in range(16):
                        for t in range(4):
                            mmb(pso[t], attnT[:, hc, t * P:(t + 1) * P],
                                w[:, hc, :], hc == 0, hc == 15)
                    for t in range(4):
                        res = p_co.tile([P, 512], f32, tag="res", name="res")
                        nc.sync.dma_start(
                            res[:], xq[t * P:(t + 1) * P,
                                       f * 512:(f + 1) * 512])
                        nc.vector.tensor_add(
                            acc[t][:, f * 512:(f + 1) * 512],
                            pso[t], res[:])
                        # incremental sumsq for the post-attn rms
                        sqy = p_cs.tile([P, 512], f32, tag="sqy",
                                        name="sqy", bufs=2)
                        nc.scalar.activation(
                            sqy[:], acc[t][:, f * 512:(f + 1) * 512],
                            AF.Square, accum_out=mq4[t][:, f:f + 1])
            s_att.close()
            s_qtp.close()

            # y = rms(h1) -> yT (bf16)
            yT = p_c.tile([P, 16, TQ], bf16, name="yT")
            for t in range(4):
                m1y = p_cs.tile([P, 1], f32, tag="m1y", name="m1y", bufs=2)
                nc.vector.tensor_reduce(m1y[:], mq4[t][:], AX.X, OP.add)
                s3 = p_cs.tile([P, 1], f32, tag="s3", name="s3", bufs=2)
                nc.scalar.activation(s3[:], m1y[:], AF.Sqrt, scale=1.0 / H,
                                     bias=epst[:])
                nc.vector.reciprocal(s3[:], s3[:])
                yn = p_cs.tile([P, H], bf16, tag="y2ksqsq", name="yn",
                               bufs=2)
                nc.vector.tensor_scalar_mul(yn[:], acc[t][:], s3[:])
                for g in range(4):
                    tr4b(yT[:, 4 * g:4 * (g + 1), t * P:(t + 1) * P],
                         [yn[:, (4 * g + k) * P:(4 * g + k + 1) * P]
                          for k in range(4)])

            # MLP phase 1: gate/up transposed -> m^T [im, tok] (bf16)
            NIC = IMP // P  # 86 im chunks
            s_mt = ExitStack()
            p_mt = s_mt.enter_context(tc.tile_pool(name="p_mt", bufs=1))
            mT = p_mt.tile([P, NIC, TQ], bf16, name="mT")
            if True:
                for ip in range(NIC // 2):
                    if ip < 2:
                        wgt, wut = pre_w[ip]
                    else:
                        wgt = wbig.tile([P, 16, 256], bf16, tag="wg",
                                        name="wgt")
                        nc.sync.dma_start(
                            wgt[:], wgr[:, :, ip * 256:(ip + 1) * 256])
                        wut = wbig.tile([P, 16, 256], bf16, tag="wu",
                                        name="wut")
                        nc.sync.dma_start(
                            wut[:], wur[:, :, ip * 256:(ip + 1) * 256])
                    for ii in range(2):
                        ic = 2 * ip + ii
                        pg, pu = pa(2)
                        for hc in range(16):
                            mmb(pg, wgt[:, hc, ii * P:(ii + 1) * P],
                                yT[:, hc, :], hc == 0, hc == 15)
                            mmb(pu, wut[:, hc, ii * P:(ii + 1) * P],
                                yT[:, hc, :], hc == 0, hc == 15)
                        gs = p_cs.tile([P, 512], bf16, tag="gs", name="gs",
                                       bufs=2)
                        nc.scalar.activation(gs[:], pg, AF.Silu)
                        nc.vector.tensor_mul(mT[:, ic, :], gs[:], pu)

                # MLP phase 2: down, psum-accumulated over all im chunks
                for f in range(4):
                    pd = [ppB.tile([P, 512], f32, tag="b", name="pd"),
                          ppB.tile([P, 512], f32, tag="b", name="pd"),
                          ppT.tile([P, 512], f32, tag="t", name="pd"),
                          ppT.tile([P, 512], f32, tag="t", name="pd")]
                    for io in range((NIC + 3) // 4):
                        nic = min(4, NIC - io * 4)
                        wdt = wbig.tile([P, 4, 512], bf16, tag="wd",
                                        name="wdt")
                        nc.sync.dma_start(
                            wdt[:, :nic, :],
                            wdr[:, io * 4:io * 4 + nic,
                                f * 512:(f + 1) * 512])
                        for ii in range(nic):
                            ic = io * 4 + ii
                            for t in range(4):
                                mmb(pd[t], mT[:, ic, t * P:(t + 1) * P],
                                    wdt[:, ii, :], ic == 0, ic == NIC - 1)
                    for t in range(4):
                        nc.vector.tensor_add(
                            acc[t][:, f * 512:(f + 1) * 512],
                            acc[t][:, f * 512:(f + 1) * 512], pd[t])
                        nc.sync.dma_start(
                            out[t * P:(t + 1) * P, f * 512:(f + 1) * 512],
                            acc[t][:, f * 512:(f + 1) * 512])
            s_mt.close()
        s_c.close()

    nc.compile()
    return nc


# ---------------------------------------------------------------------------
# host side
# ---------------------------------------------------------------------------

_DEINT = np.concatenate([np.arange(0, ROPE, 2), np.arange(1, ROPE, 2)])
BF = ml_dtypes.bfloat16


def _rmat():
    r = np.zeros((P, P), np.float32)
    for m in range(P):
        base = (m // 64) * 64
        k = base + ((m - base) + 32) % 64
        r[k, m] = 1.0
    return r


def _prep_core(c, hs, ins):
    b, j = c // 4, c % 4
    qch = [j, 4 + j, 8 + j, 12 + j]
    qrows = np.concatenate([np.arange(ch * P, (ch + 1) * P) for ch in qch])

    pos = np.asarray(ins["position_ids"])[b].astype(np.int64)
    cosg = np.asarray(ins["cos"])[pos]           # [S, 64]
    sing = np.asarray(ins["sin"])[pos]
    ssgn = np.concatenate([-sing[:, :32], sing[:, 32:]], 1)

    def dup(x):  # [S,64] -> [128, S]
        xt = np.ascontiguousarray(x.T.astype(np.float32))
        return np.concatenate([xt, xt], 0)

    iln = np.asarray(ins["input_ln_w"])[:, None]
    qln = np.asarray(ins["q_a_ln_w"])[:, None]
    kln = np.asarray(ins["kv_a_ln_w"])[:, None]
    pln = np.asarray(ins["post_ln_w"])[:, None]

    wqa = (iln * np.asarray(ins["q_a_kernel"])).astype(np.float32)
    wqb_ = (qln * np.asarray(ins["q_b_kernel"])).reshape(QLR, NH, QHD)
    wqb = np.concatenate(
        [wqb_[:, :, :NOPE].reshape(QLR, NH * NOPE),
         wqb_[:, :, NOPE:][:, :, _DEINT].reshape(QLR, NH * ROPE)], 1)
    kva = iln * np.asarray(ins["kv_a_kernel"])
    rope_d = kva[:, KVLR:][:, _DEINT]
    wkva = np.concatenate([kva[:, :KVLR], rope_d, rope_d], 1)
    wkb = (kln * np.asarray(ins["kv_b_kernel"])).reshape(KVLR, NH, NOPE + VD)
    wk = wkb[:, :, :NOPE].reshape(KVLR, NH * NOPE)
    wv = wkb[:, :, NOPE:].reshape(KVLR, NH * VD)
    wg = np.zeros((H, IMP), np.float32)
    wg[:, :IM] = pln * np.asarray(ins["gate_kernel"])
    wu = np.zeros((H, IMP), np.float32)
    wu[:, :IM] = pln * np.asarray(ins["up_kernel"])
    wd = np.zeros((IMP, H), np.float32)
    wd[:IM, :] = np.asarray(ins["down_kernel"])
    wg, wu, wd = wg.astype(BF), wu.astype(BF), wd.astype(BF)

    # masks: for key chunk kc, q chunks FQ[kc]..FQ[kc]+MW[kc]-1 get a
    # 0/NEG additive causal mask [128 keys, 128 q] each
    maskt = np.zeros((P, 16, 256), np.float32)
    pp = np.arange(P)
    for kc in range(16):
        for w in range(MW[kc]):
            qc = FQ[kc] + w  # local q chunk
            gq = (4 * qc + j) * P + pp[None, :]   # global q position
            gk = kc * P + pp[:, None]             # global k position
            maskt[:, kc, w * P:(w + 1) * P] = np.where(
                gq >= gk, np.float32(0), np.float32(NEG))

    f32c = lambda x: np.ascontiguousarray(x, dtype=np.float32)
    bfc = lambda x: np.ascontiguousarray(np.asarray(x, np.float32),
                                         dtype=BF)
    return {
        "hidb": bfc(hs[b]),
        "xqb": bfc(hs[b][qrows]),
        "xq": f32c(hs[b][qrows]),
        "wqa": bfc(wqa),
        "wqb": bfc(wqb),
        "wkva": bfc(wkva),
        "wk": bfc(wk),
        "wv": bfc(wv),
        "wo": bfc(np.asarray(ins["o_kernel"])),
        "wg": wg,
        "wu": wu,
        "wd": wd,
        "cosq": bfc(dup(cosg)[:, qrows]),
        "sinq": bfc(dup(ssgn)[:, qrows]),
        "cosk": bfc(dup(cosg)),
        "sink": bfc(dup(ssgn)),
        "maskt": maskt.reshape(P, 16 * 256),
        "rmat": _rmat(),
        "rmatb": bfc(_rmat()),
    }, qrows


def kernel(**inputs):
    from concourse import bass_utils

    hs = np.asarray(inputs["hidden_states"], dtype=np.float32)
    in_maps, qrows_l = [], []
    for c in range(8):
        m, qr = _prep_core(c, hs, inputs)
        in_maps.append(m)
        qrows_l.append(qr)

    nc = _build()
    res = bass_utils.run_bass_kernel_spmd(
        nc, in_maps, core_ids=list(range(8)))

    out = np.empty((B, S, H), np.float32)
    for c in range(8):
        out[c // 4, qrows_l[c]] = res.results[c]["out"]
    return out
